# revision 1
# baseline (speedup 1.0000x reference)
"""Multi-head attention (B=1, S=4096, D=768, H=12) on 8 Trainium2 NeuronCores.

Sharding: 4 head-groups x 2 sequence-halves. Core (g, s) computes heads
[3g, 3g+3) for query rows [2048*s, 2048*(s+1)): it projects q for its rows,
k/v for its heads over the full sequence, runs softmax(QK^T/8)V for its
(heads, rows) block, and applies its slice of the output projection. The
o-proj partials of the 4 head-groups are summed on the host (the all-reduce
step of tensor-parallel attention), halves concatenated, bias added.

On-chip layout notes:
 - scores are built transposed ([keys, queries]) so the attn@V matmul can
   contract keys on the partition axis with no transposes anywhere.
 - the head pair (h0, h1) shares the 128-row PE array via row tiling
   (K=64 each); the odd head h2 runs in rows 0-63 alone.
 - exp row-sums come for free from the attn@V matmul: V is extended with a
   65th column of ones, so PSUM row 64 accumulates sum_k exp(score).
 - softmax uses no max-subtraction: |scores| < ~30 here, safe in fp32.
"""

import numpy as np
import ml_dtypes

import concourse.bass as bass
import concourse.mybir as mybir
import concourse.tile as tile

BF16 = mybir.dt.bfloat16
FP32 = mybir.dt.float32

D = 768            # model dim
HD = 64            # head dim
HPC = 3            # heads per core
DH = HPC * HD      # 192: head dims per core
SEQ = 4096         # full sequence (keys)
SQ = 2048          # query rows per core
CT = D // 128      # 6 contraction tiles for projections
QB = 512           # query block (matmul free dim)
NQB = SQ // QB     # 4
KBLK = 128         # key block (PSUM partition dim)
NKB = SEQ // KBLK  # 32
KT = 512           # k/v load superblock
NKT = SEQ // KT    # 8
SCALE = 1.0 / 8.0  # 1/sqrt(HD)


def _patch_tile_drain():
    """walrus here accepts only one sync-wait per CTRL instruction; the stock
    TileContext exit packs every outstanding wait onto a single SP Drain.
    Split them onto single-wait SP NOPs that precede the drain."""
    import bass_rust
    from concourse.vector_clock import ScopedClock

    def _split_drain_and_barrier(self, tick_clock, wait_clock):
        nc = self.nc
        probe = nc.sync.nop(nofuse=True)
        wait_clock.add_sem_waits(
            probe.ins, ScopedClock({None: tick_clock.global_clock})
        )
        si = probe.ins.sync_info
        waits = list(si.on_wait) if si is not None and si.on_wait else []
        if len(waits) > 1:
            probe.ins.sync_info = bass_rust.SyncInfo(
                on_wait=[waits[0]], on_update=[]
            )
            for w in waits[1:]:
                n = nc.sync.nop(nofuse=True)
                n.ins.sync_info = bass_rust.SyncInfo(on_wait=[w], on_update=[])
        nc.sync.drain()
        nc.all_engine_barrier()
        assert self.sems is not None
        popped = nc._tile_sem_poison_stack.pop()
        assert popped is self._sem_poison
        nc.clear_and_free_semaphores(list(self.sems.allocated().values()))
        nc.all_engine_barrier()

    tile.TileContext._drain_and_barrier = _split_drain_and_barrier



def _split_multi_waits(nc):
    """Hoist all-but-one sync-waits of every instruction onto preceding
    single-wait NOPs on the same engine (walrus 1-wait limit)."""
    import bass_rust
    n_split = 0
    for bb in nc.main_func.blocks:
        insts = bb.instructions
        new_list = []
        for inst in insts:
            si = getattr(inst, "sync_info", None)
            if si is not None and si.on_wait and len(si.on_wait) > 1:
                waits = list(si.on_wait)
                n_split += 1
                for w in waits[:-1]:
                    nop = mybir.InstNoOp(
                        name=nc.get_next_instruction_name(),
                        engine=inst.engine, ins=[], outs=[],
                        sync_info=bass_rust.SyncInfo(
                            on_wait=[w], on_update=[]))
                    new_list.append(nop)
                inst.sync_info = bass_rust.SyncInfo(
                    on_wait=[waits[-1]], on_update=list(si.on_update))
            new_list.append(inst)
        insts[:] = new_list
    return n_split

def build_program(has_bq: bool, has_bk: bool, has_bv: bool,
                  repeat: int = 1, qk_dtype=BF16) -> bass.Bass:
    _patch_tile_drain()
    nc = bass.Bass()

    qTs = nc.dram_tensor("qTs", [D, SQ], BF16, kind="ExternalInput")
    kT = nc.dram_tensor("kT", [D, SEQ], BF16, kind="ExternalInput")
    vT = nc.dram_tensor("vT", [D, SEQ], BF16, kind="ExternalInput")
    wq = nc.dram_tensor("wq", [D, DH], BF16, kind="ExternalInput")
    wk = nc.dram_tensor("wk", [D, DH], BF16, kind="ExternalInput")
    wv = nc.dram_tensor("wv", [D, DH], BF16, kind="ExternalInput")
    wo = nc.dram_tensor("wo", [DH, D], BF16, kind="ExternalInput")
    bqd = nc.dram_tensor("bq", [DH, 1], FP32, kind="ExternalInput")
    bkd = nc.dram_tensor("bk", [DH, 1], FP32, kind="ExternalInput")
    bvd = nc.dram_tensor("bv", [DH, 1], FP32, kind="ExternalInput")
    outT = nc.dram_tensor("outT", [D, SQ], FP32, kind="ExternalOutput")

    Exp = mybir.ActivationFunctionType.Exp

    with tile.TileContext(nc) as tc:
        with (
            tc.tile_pool(name="persist", bufs=1) as persist,
            tc.tile_pool(name="small", bufs=2) as small,
        ):
            # persistent SBUF tensors
            khT_pair = persist.tile([128, SEQ], qk_dtype, tag="khp", name="khp")
            khT_h2 = persist.tile([64, SEQ], qk_dtype, tag="kh2", name="kh2")
            qhT_pair = persist.tile([128, SQ], qk_dtype, tag="qhp", name="qhp")
            qhT_h2 = persist.tile([64, SQ], qk_dtype, tag="qh2", name="qh2")
            vhx = [persist.tile([128, NKB * 65], BF16, tag=f"vhx{h}", name=f"vhx{h}")
                   for h in range(HPC)]
            wq_sb = persist.tile([128, CT * DH], BF16, tag="wq", name="wq_sb")
            wk_sb = persist.tile([128, CT * DH], BF16, tag="wk", name="wk_sb")
            wv_sb = persist.tile([128, CT * DH], BF16, tag="wv", name="wv_sb")
            wo_sb1 = persist.tile([128, D], BF16, tag="wo1", name="wo1")
            wo_sb2 = persist.tile([64, D], BF16, tag="wo2", name="wo2")
            bq_sb = persist.tile([128, 1], FP32, tag="bq1", name="bq1")
            bq2_sb = persist.tile([64, 1], FP32, tag="bq2", name="bq2")
            bk_sb = persist.tile([128, 1], FP32, tag="bk1", name="bk1")
            bk2_sb = persist.tile([64, 1], FP32, tag="bk2", name="bk2")
            bv_sb = persist.tile([64, HPC], FP32, tag="bv", name="bv_sb")
            ones_sb = persist.tile([1, 64], FP32, tag="ones", name="ones_sb")

            # ones columns for the exp-sum trick (overwritten with vh below)
            for h in range(HPC):
                nc.gpsimd.memset(vhx[h][:], 1.0)
            nc.vector.memset(ones_sb[:], 1.0)

            persist_tiles = (khT_pair, khT_h2, qhT_pair, qhT_h2, vhx,
                             wq_sb, wk_sb, wv_sb, wo_sb1, wo_sb2,
                             bq_sb, bq2_sb, bk_sb, bk2_sb, bv_sb, ones_sb,
                             qTs, kT, vT, outT,
                             wq, wk, wv, wo, bqd, bkd, bvd)
            for _rep in range(repeat):
                _phases(nc, tc, has_bq, has_bk, has_bv, persist_tiles, small)
    _split_multi_waits(nc)
    return nc


def _phases(nc, tc, has_bq, has_bk, has_bv, P, small):
    (khT_pair, khT_h2, qhT_pair, qhT_h2, vhx, wq_sb, wk_sb, wv_sb,
     wo_sb1, wo_sb2, bq_sb, bq2_sb, bk_sb, bk2_sb, bv_sb, ones_sb,
     qTs, kT, vT, outT, wq, wk, wv, wo, bqd, bkd, bvd) = P
    Exp = mybir.ActivationFunctionType.Exp

    def psum_to_sbuf(dst_ap, src_ap, bias_ap):
        if bias_ap is None:
            nc.vector.tensor_copy(dst_ap, src_ap)
        else:
            nc.vector.tensor_scalar_add(dst_ap, src_ap, bias_ap)

    def scores_mms(ps_ap, h, kb, q0, width):
        """scores^T[kb block, q0:q0+width] for head h into PSUM ap."""
        ks = slice(kb * KBLK, (kb + 1) * KBLK)
        if h == 0:
            lhs, rhs = khT_pair[0:64, ks], qhT_pair[0:64, q0:q0 + width]
        elif h == 1:
            lhs, rhs = khT_pair[64:128, ks], qhT_pair[64:128, q0:q0 + width]
        else:
            lhs, rhs = khT_h2[:, ks], qhT_h2[:, q0:q0 + width]
        nc.tensor.matmul(ps_ap, lhs, rhs, start=True, stop=True)

    def normalize_oproj(accs, q0, attnsb, accpool, outsb, tag="acc",
                        tbufs=None):
        attn_pair = attnsb.tile([128, QB], BF16, tag="apair", name="apair")
        attn_h2 = attnsb.tile([64, QB], BF16, tag="ah2", name="ah2")
        for h in range(HPC):
            sums = small.tile([1, QB], FP32, tag="sums", name="sums")
            nc.vector.tensor_copy(sums[:], accs[h][64:65, :])
            rb_ps = accpool.tile([64, QB], FP32, tag=tag, name="rb_ps",
                                 bufs=tbufs)
            nc.tensor.matmul(rb_ps[:], ones_sb[:], sums[:],
                             start=True, stop=True)
            rb = small.tile([64, QB], FP32, tag="rb", name="rb")
            nc.vector.reciprocal(rb[:], rb_ps[:])
            dst = (attn_pair[h * 64:(h + 1) * 64, :]
                   if h < 2 else attn_h2[:])
            nc.vector.tensor_mul(dst, accs[h][0:64, :], rb[:])
            if has_bv:
                nc.vector.tensor_scalar_add(dst, dst, bv_sb[:, h:h + 1])
        for et in range(CT):
            e0 = et * 128
            pso = accpool.tile([128, QB], FP32, tag=tag, name="pso",
                               bufs=tbufs)
            nc.tensor.matmul(pso[:], wo_sb1[:, e0:e0 + 128],
                             attn_pair[:], start=True, stop=False)
            nc.tensor.matmul(pso[:], wo_sb2[:, e0:e0 + 128],
                             attn_h2[:], start=False, stop=True)
            osb = outsb.tile([128, QB], FP32, tag="osb", name="osb")
            nc.vector.tensor_copy(osb[:], pso[:])
            nc.sync.dma_start(outT[e0:e0 + 128, q0:q0 + QB], osb[:])

    # weight loads, ordered to unblock the pipeline front-to-back
    for ct in range(CT):
        nc.sync.dma_start(wq_sb[:, ct * DH:(ct + 1) * DH],
                          wq[ct * 128:ct * 128 + 128, :])
    if has_bq:
        nc.sync.dma_start(bq_sb[:], bqd[0:128, :])
        nc.sync.dma_start(bq2_sb[:], bqd[128:DH, :])

    def load_wkv():
        for ct in range(CT):
            c0 = ct * 128
            nc.sync.dma_start(wk_sb[:, ct * DH:(ct + 1) * DH],
                              wk[c0:c0 + 128, :])
            nc.sync.dma_start(wv_sb[:, ct * DH:(ct + 1) * DH],
                              wv[c0:c0 + 128, :])
        if has_bk:
            nc.sync.dma_start(bk_sb[:], bkd[0:128, :])
            nc.sync.dma_start(bk2_sb[:], bkd[128:DH, :])

    def load_wo():
        nc.sync.dma_start(wo_sb1[:], wo[0:128, :])
        nc.sync.dma_start(wo_sb2[:], wo[128:DH, :])
        if has_bv:
            for h in range(HPC):
                nc.sync.dma_start(bv_sb[:, h:h + 1],
                                  bvd[h * HD:(h + 1) * HD, :])

    # ---- Phase A+B0: projections interleaved with attention for qb 0 ----
    # PSUM budget (8 banks): pk/pk2/pv share a 3-bank projection set,
    # qb0 scores 2 banks, qb0 accumulators 3 banks.
    with (
        tc.tile_pool(name="acc0", bufs=1, space="PSUM") as acc0_pool,
        tc.tile_pool(name="pt0", bufs=6) as pt0_pool,
        tc.tile_pool(name="attnsb", bufs=2) as attnsb,
        tc.tile_pool(name="outsb", bufs=3) as outsb,
      ):
      accs0 = [acc0_pool.tile([128, QB], FP32, tag=f"a0{h}", name="a0",
                              bufs=1)
               for h in range(HPC)]
      with (
        tc.tile_pool(name="stream", bufs=2) as stream,
        tc.tile_pool(name="pproj", bufs=1, space="PSUM") as pproj,
        tc.tile_pool(name="sc0", bufs=2, space="PSUM") as sc0_pool,
      ):
        # q projection (all four query blocks)
        qt2_tiles = []
        for st in range(NQB):
            s0 = st * QB
            ps_q = pproj.tile([128, QB], FP32, tag="pk", name="psq")
            ps_q2 = pproj.tile([64, QB], FP32, tag="pk2", name="psq2")
            if st % 2 == 0:
                qt2_tiles = []
                for ct in range(CT):
                    t = stream.tile([128, 2 * QB], BF16, tag="qt", name="qt",
                                    bufs=12)
                    nc.sync.dma_start(
                        t[:], qTs[ct * 128:(ct + 1) * 128, s0:s0 + 2 * QB])
                    qt2_tiles.append(t)
            qhalf = slice((st % 2) * QB, (st % 2) * QB + QB)
            qt_tiles = [t[:, qhalf] for t in qt2_tiles]
            for ct in range(CT):
                nc.tensor.matmul(
                    ps_q[:], wq_sb[:, ct * DH:ct * DH + 128], qt_tiles[ct][:],
                    start=(ct == 0), stop=(ct == CT - 1))
            for ct in range(CT):
                nc.tensor.matmul(
                    ps_q2[:], wq_sb[:, ct * DH + 128:(ct + 1) * DH],
                    qt_tiles[ct][:],
                    start=(ct == 0), stop=(ct == CT - 1))
            psum_to_sbuf(qhT_pair[:, s0:s0 + QB], ps_q[:],
                         bq_sb[:, 0:1] if has_bq else None)
            psum_to_sbuf(qhT_h2[:, s0:s0 + QB], ps_q2[:],
                         bq2_sb[:, 0:1] if has_bq else None)
            if st == 0:
                load_wkv()

        kt2_tiles = {}
        for kt in range(NKT):
            k0 = kt * KT
            if kt == 2:
                load_wo()
            # k/v loads come in 1024-wide tiles (2KB partition lines);
            # each serves two 512-key superblocks.
            if kt % 2 == 0:
                kw, vw = [], []
                for ct in range(CT):
                    c0 = ct * 128
                    t = stream.tile([128, 2 * KT], BF16, tag="ktile",
                                    name="ktile", bufs=12)
                    nc.sync.dma_start(t[:], kT[c0:c0 + 128, k0:k0 + 2 * KT])
                    kw.append(t)
                    t = stream.tile([128, 2 * KT], BF16, tag="vtile",
                                    name="vtile", bufs=12)
                    nc.sync.dma_start(t[:], vT[c0:c0 + 128, k0:k0 + 2 * KT])
                    vw.append(t)
                kt2_tiles = {"k": kw, "v": vw}
            half = slice((kt % 2) * KT, (kt % 2) * KT + KT)
            kt_tiles = [t[:, half] for t in kt2_tiles["k"]]
            vt_tiles = [t[:, half] for t in kt2_tiles["v"]]
            ps_kh = pproj.tile([128, KT], FP32, tag="pk", name="pskh")
            ps_kh2 = pproj.tile([64, KT], FP32, tag="pk2", name="pskh2")
            for ct in range(CT):
                nc.tensor.matmul(
                    ps_kh[:], wk_sb[:, ct * DH:ct * DH + 128],
                    kt_tiles[ct][:], start=(ct == 0), stop=(ct == CT - 1))
            for ct in range(CT):
                nc.tensor.matmul(
                    ps_kh2[:], wk_sb[:, ct * DH + 128:(ct + 1) * DH],
                    kt_tiles[ct][:], start=(ct == 0), stop=(ct == CT - 1))
            psum_to_sbuf(khT_pair[:, k0:k0 + KT], ps_kh[:],
                         bk_sb[:, 0:1] if has_bk else None)
            psum_to_sbuf(khT_h2[:, k0:k0 + KT], ps_kh2[:],
                         bk2_sb[:, 0:1] if has_bk else None)
            for sj in range(KT // KBLK):
                kb = kt * (KT // KBLK) + sj
                ps_vh = pproj.tile([128, DH], FP32, tag="pv", name="psvh")
                for ct in range(CT):
                    nc.tensor.matmul(
                        ps_vh[:], vt_tiles[ct][:, sj * KBLK:(sj + 1) * KBLK],
                        wv_sb[:, ct * DH:(ct + 1) * DH],
                        start=(ct == 0), stop=(ct == CT - 1))
                for h in range(HPC):
                    nc.vector.tensor_copy(
                        vhx[h][:, kb * 65:kb * 65 + 64],
                        ps_vh[:, h * HD:(h + 1) * HD])
                # attention for query block 0 on this key block
                for h in range(HPC):
                    sc = sc0_pool.tile([128, QB], FP32, tag="sc0", name="sc0")
                    scores_mms(sc[:], h, kb, 0, QB)
                    pt = pt0_pool.tile([128, QB], BF16, tag="pt0", name="pt0")
                    nc.scalar.activation(pt[:], sc[:], Exp, scale=SCALE)
                    nc.tensor.matmul(
                        accs0[h][0:65, :], vhx[h][:, kb * 65:kb * 65 + 65],
                        pt[:], start=(kb == 0), stop=(kb == NKB - 1))
      with tc.tile_pool(name="pfin", bufs=2, space="PSUM") as pfin:
        normalize_oproj(accs0, 0, attnsb, pfin, outsb, tag="fin", tbufs=2)

    # ---- Phase B: attention + o-proj for query blocks 1..3 ----
    with (
        tc.tile_pool(name="scpool", bufs=2, space="PSUM") as scpool,
        tc.tile_pool(name="accpool", bufs=4, space="PSUM") as accpool,
        tc.tile_pool(name="ptpool", bufs=8) as ptpool,
        tc.tile_pool(name="attnsb", bufs=2) as attnsb,
        tc.tile_pool(name="outsb", bufs=3) as outsb,
    ):
        for qb in range(1, NQB):
            q0 = qb * QB
            accs = [accpool.tile([128, QB], FP32, tag="acc", name="acc")
                    for _ in range(HPC)]
            for kb2 in range(NKB // 2):
                pts = []
                for h in range(HPC):
                    ps = scpool.tile([128, 2 * QB], FP32, tag="sc", name="sc")
                    for j in range(2):
                        kb = kb2 * 2 + j
                        scores_mms(ps[:, j * QB:(j + 1) * QB], h, kb, q0, QB)
                    pt = ptpool.tile([128, 2 * QB], BF16, tag="pt", name="pt")
                    nc.scalar.activation(pt[:], ps[:], Exp, scale=SCALE)
                    pts.append(pt)
                for h in range(HPC):
                    for j in range(2):
                        kb = kb2 * 2 + j
                        nc.tensor.matmul(
                            accs[h][0:65, :],
                            vhx[h][:, kb * 65:kb * 65 + 65],
                            pts[h][:, j * QB:(j + 1) * QB],
                            start=(kb == 0), stop=(kb == NKB - 1))
            normalize_oproj(accs, q0, attnsb, accpool, outsb)


def prepare(q, k, v, Wq, bq, Wk, bk, Wv, bv, Wo, bo):
    """Host-side sharding: returns (in_maps for cores 0-7, bias flags)."""
    bf = ml_dtypes.bfloat16
    qT = np.ascontiguousarray(q[0].T).astype(bf)
    kTf = np.ascontiguousarray(k[0].T).astype(bf)
    vTf = np.ascontiguousarray(v[0].T).astype(bf)
    wqT = np.ascontiguousarray(np.asarray(Wq).T).astype(bf)
    wkT = np.ascontiguousarray(np.asarray(Wk).T).astype(bf)
    wvT = np.ascontiguousarray(np.asarray(Wv).T).astype(bf)
    woT = np.ascontiguousarray(np.asarray(Wo).T).astype(bf)
    bq = np.asarray(bq, np.float32)
    bk = np.asarray(bk, np.float32)
    bv = np.asarray(bv, np.float32)
    in_maps = []
    for core in range(8):
        g, s = divmod(core, 2)
        d0, d1 = g * DH, (g + 1) * DH
        in_maps.append({
            "qTs": np.ascontiguousarray(qT[:, s * SQ:(s + 1) * SQ]),
            "kT": kTf,
            "vT": vTf,
            "wq": np.ascontiguousarray(wqT[:, d0:d1]),
            "wk": np.ascontiguousarray(wkT[:, d0:d1]),
            "wv": np.ascontiguousarray(wvT[:, d0:d1]),
            "wo": np.ascontiguousarray(woT[d0:d1, :]),
            "bq": np.ascontiguousarray(bq[d0:d1]).reshape(DH, 1),
            "bk": np.ascontiguousarray(bk[d0:d1]).reshape(DH, 1),
            "bv": np.ascontiguousarray(bv[d0:d1]).reshape(DH, 1),
        })
    flags = (bool(np.any(bq)), bool(np.any(bk)), bool(np.any(bv)))
    return in_maps, flags


def combine(results, bo):
    """Host-side unsharding: sum o-proj partials per half, concat, add bo."""
    halves = []
    for s in range(2):
        acc = None
        for g in range(4):
            o = np.asarray(results[g * 2 + s]["outT"], np.float32)
            acc = o if acc is None else acc + o
        halves.append(acc.T)
    out = np.concatenate(halves, axis=0) + np.asarray(bo, np.float32)
    return np.ascontiguousarray(out).reshape(1, SEQ, D).astype(np.float32)


def kernel(q, k, v, Wq, bq, Wk, bk, Wv, bv, Wo, bo):
    from concourse.bass_utils import run_bass_kernel_spmd

    in_maps, flags = prepare(q, k, v, Wq, bq, Wk, bk, Wv, bv, Wo, bo)
    nc = build_program(*flags)
    last_err = None
    for _attempt in range(3):
        try:
            res = run_bass_kernel_spmd(nc, in_maps, list(range(8)))
            return combine(res.results, bo)
        except Exception as e:  # transient NRT/device wedges recover on retry
            last_err = e
            try:
                import jax
                jax.clear_caches()
                jax.extend.backend.clear_backends()
            except Exception:
                pass
    raise last_err



# revision 11
# speedup vs baseline: 1.9427x; 1.9427x over previous
"""Multi-head attention (B=1, S=4096, D=768, H=12) on 8 Trainium2 NeuronCores.

Sharding: 4 head-groups x 2 sequence-halves. Core (g, s) computes heads
[3g, 3g+3) for query rows [2048*s, 2048*(s+1)): it projects q for its rows,
k/v for its heads over the full sequence, runs softmax(QK^T/8)V for its
(heads, rows) block, and applies its slice of the output projection. The
o-proj partials of the 4 head-groups are summed on the host (the all-reduce
step of tensor-parallel attention), halves concatenated, bias added.

On-chip layout notes:
 - scores are built transposed ([keys, queries]) so the attn@V matmul can
   contract keys on the partition axis with no transposes anywhere.
 - PE-array tiling: the head pair (h0, h1) shares the array via row tiling
   (K=64 each); the odd head h2 pairs its even/odd key blocks the same way
   (khT/qhT for h2 are duplicated into partitions 64-127). The attn@V
   matmuls for h0/h1 run as col-tiled M=64 pairs; exp row-sums come from
   dedicated ones[128,32] col tiles packed into spare 32-col strips.
 - softmax uses no max-subtraction: |scores| < ~30 here, safe in fp32.
 - exp is split across engines: most tiles use the scalar engine's LUT
   exp; a rotating subset is computed on the vector engine / gpsimd as a
   Schraudolph-style bf16 bit-trick (out_i16 = round(score*A + B) IS the
   bf16 pattern of exp(score/8)), trading ~1.8% rms error on those tiles
   for removing the scalar engine from the critical path.
"""

import numpy as np
import ml_dtypes

import concourse.bass as bass
import concourse.mybir as mybir
import concourse.tile as tile

BF16 = mybir.dt.bfloat16
FP32 = mybir.dt.float32
I16 = mybir.dt.int16

D = 768            # model dim
HD = 64            # head dim
HPC = 3            # heads per core
DH = HPC * HD      # 192: head dims per core
SEQ = 4096         # full sequence (keys)
SQ = 2048          # query rows per core
CT = D // 128      # 6 contraction tiles for projections
QB = 512           # query block (matmul free dim)
NQB = SQ // QB     # 4
KBLK = 128         # key block (PSUM partition dim)
NKB = SEQ // KBLK  # 32
KT = 512           # k/v load superblock
NKT = SEQ // KT    # 8
SCALE = 1.0 / 8.0  # 1/sqrt(HD)

# Schraudolph bf16 exp: bits16 = round(raw_score * A_S + B_S) is the bf16
# pattern of ~exp(raw_score/8). B offset 0.0579 zero-means the rel error.
LOG2E = 1.4426950408889634
A_S = 128.0 * LOG2E * SCALE
B_S = 127.0 * 128.0 - 128.0 * 0.0579
MULT = mybir.AluOpType.mult
ADD = mybir.AluOpType.add


def _patch_tile_drain():
    """walrus here accepts only one sync-wait per CTRL instruction; the stock
    TileContext exit packs every outstanding wait onto a single SP Drain.
    Split them onto single-wait SP NOPs that precede the drain."""
    import bass_rust
    from concourse.vector_clock import ScopedClock

    def _split_drain_and_barrier(self, tick_clock, wait_clock):
        nc = self.nc
        probe = nc.sync.nop(nofuse=True)
        wait_clock.add_sem_waits(
            probe.ins, ScopedClock({None: tick_clock.global_clock})
        )
        si = probe.ins.sync_info
        waits = list(si.on_wait) if si is not None and si.on_wait else []
        if len(waits) > 1:
            probe.ins.sync_info = bass_rust.SyncInfo(
                on_wait=[waits[0]], on_update=[]
            )
            for w in waits[1:]:
                n = nc.sync.nop(nofuse=True)
                n.ins.sync_info = bass_rust.SyncInfo(on_wait=[w], on_update=[])
        nc.sync.drain()
        nc.all_engine_barrier()
        assert self.sems is not None
        popped = nc._tile_sem_poison_stack.pop()
        assert popped is self._sem_poison
        nc.clear_and_free_semaphores(list(self.sems.allocated().values()))
        nc.all_engine_barrier()

    tile.TileContext._drain_and_barrier = _split_drain_and_barrier


def _split_multi_waits(nc):
    """Hoist all-but-one sync-waits of every instruction onto preceding
    single-wait NOPs on the same engine (walrus 1-wait limit)."""
    import bass_rust
    n_split = 0
    for bb in nc.main_func.blocks:
        insts = bb.instructions
        new_list = []
        for inst in insts:
            si = getattr(inst, "sync_info", None)
            if si is not None and si.on_wait and len(si.on_wait) > 1:
                waits = list(si.on_wait)
                n_split += 1
                for w in waits[:-1]:
                    nop = mybir.InstNoOp(
                        name=nc.get_next_instruction_name(),
                        engine=inst.engine, ins=[], outs=[],
                        sync_info=bass_rust.SyncInfo(
                            on_wait=[w], on_update=[]))
                    new_list.append(nop)
                inst.sync_info = bass_rust.SyncInfo(
                    on_wait=[waits[-1]], on_update=list(si.on_update))
            new_list.append(inst)
        insts[:] = new_list
    return n_split


class ExpSched:
    """Rotates exp tiles between ACT (exact LUT exp) and the DVE
    (Schraudolph bit-trick). GpSimd cannot read PSUM, so it gets none.
    dve_r picks residues of a mod-`mod` counter."""

    def __init__(self, nc, mod=7, dve_r=(2, 5)):
        self.nc = nc
        self.u = 0
        self.mod = mod
        self.dve_r = dve_r
        self.Exp = mybir.ActivationFunctionType.Exp

    def exp(self, pt_ap, sc_ap):
        r = self.u % self.mod
        self.u += 1
        if r in self.dve_r:
            self.nc.vector.tensor_scalar(
                pt_ap.bitcast(I16), sc_ap, A_S, B_S, MULT, ADD)
        else:
            self.nc.scalar.activation(pt_ap, sc_ap, self.Exp, scale=SCALE)


def build_program(has_bq: bool, has_bk: bool, has_bv: bool,
                  repeat: int = 1, qk_dtype=BF16) -> bass.Bass:
    _patch_tile_drain()
    nc = bass.Bass()

    qTs = nc.dram_tensor("qTs", [D, SQ], BF16, kind="ExternalInput")
    kT = nc.dram_tensor("kT", [D, SEQ], BF16, kind="ExternalInput")
    vT = nc.dram_tensor("vT", [D, SEQ], BF16, kind="ExternalInput")
    wq = nc.dram_tensor("wq", [D, DH], BF16, kind="ExternalInput")
    wk = nc.dram_tensor("wk", [D, DH], BF16, kind="ExternalInput")
    wv = nc.dram_tensor("wv", [D, DH], BF16, kind="ExternalInput")
    wo = nc.dram_tensor("wo", [DH, D], BF16, kind="ExternalInput")
    bqd = nc.dram_tensor("bq", [DH, 1], FP32, kind="ExternalInput")
    bkd = nc.dram_tensor("bk", [DH, 1], FP32, kind="ExternalInput")
    bvd = nc.dram_tensor("bv", [DH, 1], FP32, kind="ExternalInput")
    outT = nc.dram_tensor("outT", [D, SQ], FP32, kind="ExternalOutput")

    with tile.TileContext(nc) as tc:
        with (
            tc.tile_pool(name="persist", bufs=1) as persist,
            tc.tile_pool(name="small", bufs=2) as small,
        ):
            # persistent SBUF tensors
            khT_pair = persist.tile([128, SEQ], qk_dtype, tag="khp", name="khp")
            khT_h2 = persist.tile([128, SEQ], qk_dtype, tag="kh2", name="kh2")
            qhT_pair = persist.tile([128, SQ], qk_dtype, tag="qhp", name="qhp")
            qhT_h2 = persist.tile([128, SQ], qk_dtype, tag="qh2", name="qh2")
            vh = [persist.tile([128, NKB * HD], BF16, tag=f"vh{h}",
                               name=f"vh{h}") for h in range(HPC)]
            ones32 = persist.tile([128, 32], BF16, tag="ones32", name="ones32")
            wq_sb = persist.tile([128, CT * DH], BF16, tag="wq", name="wq_sb")
            wk_sb = persist.tile([128, CT * DH], BF16, tag="wk", name="wk_sb")
            wv_sb = persist.tile([128, CT * DH], BF16, tag="wv", name="wv_sb")
            wo_sb1 = persist.tile([128, D], BF16, tag="wo1", name="wo1")
            wo_sb2d = persist.tile([128, D], BF16, tag="wo2", name="wo2")
            bq_sb = persist.tile([128, 1], FP32, tag="bq1", name="bq1")
            bq2_sb = persist.tile([64, 1], FP32, tag="bq2", name="bq2")
            bk_sb = persist.tile([128, 1], FP32, tag="bk1", name="bk1")
            bk2_sb = persist.tile([64, 1], FP32, tag="bk2", name="bk2")
            bv_sb = persist.tile([64, HPC], FP32, tag="bv", name="bv_sb")

            nc.vector.memset(ones32[:], 1.0)

            persist_tiles = (khT_pair, khT_h2, qhT_pair, qhT_h2, vh, ones32,
                             wq_sb, wk_sb, wv_sb, wo_sb1, wo_sb2d,
                             bq_sb, bq2_sb, bk_sb, bk2_sb, bv_sb,
                             qTs, kT, vT, outT,
                             wq, wk, wv, wo, bqd, bkd, bvd)
            for _rep in range(repeat):
                _phases(nc, tc, has_bq, has_bk, has_bv, persist_tiles, small)
    _split_multi_waits(nc)
    return nc


def _phases(nc, tc, has_bq, has_bk, has_bv, P, small):
    (khT_pair, khT_h2, qhT_pair, qhT_h2, vh, ones32, wq_sb, wk_sb, wv_sb,
     wo_sb1, wo_sb2d, bq_sb, bq2_sb, bk_sb, bk2_sb, bv_sb,
     qTs, kT, vT, outT, wq, wk, wv, wo, bqd, bkd, bvd) = P

    es = ExpSched(nc)

    def psum_to_sbuf(dst_ap, src_ap, bias_ap):
        if bias_ap is None:
            nc.vector.tensor_copy(dst_ap, src_ap)
        else:
            nc.vector.tensor_scalar_add(dst_ap, src_ap, bias_ap)

    def score_mm(ps_ap, h, kb, q0, width, odd=False):
        """scores^T[kb block, q0:q0+width] for head h into PSUM ap.
        For h==2, odd=True uses the duplicated rows 64:128 so even/odd kb
        matmuls row-tile concurrently."""
        ks = slice(kb * KBLK, (kb + 1) * KBLK)
        qs = slice(q0, q0 + width)
        if h == 0:
            lhs, rhs = khT_pair[0:64, ks], qhT_pair[0:64, qs]
        elif h == 1:
            lhs, rhs = khT_pair[64:128, ks], qhT_pair[64:128, qs]
        elif not odd:
            lhs, rhs = khT_h2[0:64, ks], qhT_h2[0:64, qs]
        else:
            lhs, rhs = khT_h2[64:128, ks], qhT_h2[64:128, qs]
        nc.tensor.matmul(ps_ap, lhs, rhs, start=True, stop=True)

    def attnv(kb, pt_aps, accA, accB, s2_ap, s2_pos, first, last,
              s2_first, s2_last):
        """attn@V + exp-sums for one key block. pt_aps = [h0, h1, h2] exp
        score APs [128, 512]. Col-tiled: h0/h1 attn pair in accA, h2 attn +
        h0/h1 sums in accB, h2 sums in s2_ap[64:96] (a [128,512] psum tile).
        """
        ks = slice(kb * HD, (kb + 1) * HD)
        nc.tensor.matmul(accA[0:64, :], vh[0][:, ks], pt_aps[0],
                         start=first, stop=last)
        nc.tensor.matmul(accA[64:128, :], vh[1][:, ks], pt_aps[1],
                         start=first, stop=last)
        nc.tensor.matmul(accB[0:64, :], vh[2][:, ks], pt_aps[2],
                         start=first, stop=last)
        nc.tensor.matmul(accB[64:96, :], ones32[:], pt_aps[0],
                         start=first, stop=last)
        nc.tensor.matmul(accB[96:128, :], ones32[:], pt_aps[1],
                         start=first, stop=last, tile_position=(0, 96))
        nc.tensor.matmul(s2_ap, ones32[:], pt_aps[2],
                         start=s2_first, stop=s2_last,
                         tile_position=(0, s2_pos))

    def normalize_oproj(q0, accA, accB, s2_aps, attnsb, psopool, outsb):
        """softmax-normalize from the sums strips, then o-proj."""
        # h2 sums: single accumulator (phase A) or even+odd pair (phase B)
        if len(s2_aps) == 1:
            s2_src = s2_aps[0]
        else:
            # DVE reads at most one PSUM operand: stage one side in SBUF
            s2c = small.tile([32, QB], FP32, tag="s2c", name="s2c")
            nc.vector.tensor_copy(s2c[:], s2_aps[1])
            s2s = small.tile([32, QB], FP32, tag="s2s", name="s2s")
            nc.vector.tensor_add(s2s[:], s2_aps[0], s2c[:])
            s2_src = s2s[:]
        r0 = small.tile([32, QB], FP32, tag="r0", name="r0")
        r1 = small.tile([32, QB], FP32, tag="r1", name="r1")
        r2 = small.tile([32, QB], FP32, tag="r2", name="r2")
        nc.vector.reciprocal(r0[:], accB[64:96, :])
        nc.vector.reciprocal(r1[:], accB[96:128, :])
        nc.vector.reciprocal(r2[:], s2_src)

        attn_pair = attnsb.tile([128, QB], BF16, tag="apair", name="apair")
        attn_h2d = attnsb.tile([128, QB], BF16, tag="ah2d", name="ah2d")
        ve = nc.vector
        ve.tensor_mul(attn_pair[0:32, :], accA[0:32, :], r0[:])
        ve.tensor_mul(attn_pair[32:64, :], accA[32:64, :], r0[:])
        ve.tensor_mul(attn_pair[64:96, :], accA[64:96, :], r1[:])
        ve.tensor_mul(attn_pair[96:128, :], accA[96:128, :], r1[:])
        # h2 attn, duplicated into rows 64:128 (via SBUF-to-SBUF DMA) so the
        # o-proj K=64 chains can row-tile across even/odd embedding tiles
        ve.tensor_mul(attn_h2d[0:32, :], accB[0:32, :], r2[:])
        ve.tensor_mul(attn_h2d[32:64, :], accB[32:64, :], r2[:])
        nc.sync.dma_start(attn_h2d[64:128, :], attn_h2d[0:64, :])
        if has_bv:
            for h, dst in ((0, attn_pair[0:64, :]), (1, attn_pair[64:128, :]),
                           (2, attn_h2d[0:64, :]), (2, attn_h2d[64:128, :])):
                nc.vector.tensor_scalar_add(dst, dst, bv_sb[:, h:h + 1])

        for ep in range(CT // 2):
            e0 = 2 * ep * 128
            pso_a = psopool.tile([128, QB], FP32, tag="scr", name="pso_a")
            nc.tensor.matmul(pso_a[:], wo_sb1[:, e0:e0 + 128],
                             attn_pair[:], start=True, stop=False)
            pso_b = psopool.tile([128, QB], FP32, tag="scr", name="pso_b")
            nc.tensor.matmul(pso_b[:], wo_sb1[:, e0 + 128:e0 + 256],
                             attn_pair[:], start=True, stop=False)
            nc.tensor.matmul(pso_a[:], wo_sb2d[0:64, e0:e0 + 128],
                             attn_h2d[0:64, :], start=False, stop=True)
            nc.tensor.matmul(pso_b[:], wo_sb2d[64:128, e0 + 128:e0 + 256],
                             attn_h2d[64:128, :], start=False, stop=True)
            for j, pso in ((0, pso_a), (1, pso_b)):
                osb = outsb.tile([128, QB], FP32, tag="osb", name="osb")
                nc.vector.tensor_copy(osb[:], pso[:])
                nc.sync.dma_start(
                    outT[e0 + j * 128:e0 + j * 128 + 128, q0:q0 + QB], osb[:])

    # weight loads, ordered to unblock the pipeline front-to-back
    for ct in range(CT):
        nc.sync.dma_start(wq_sb[:, ct * DH:(ct + 1) * DH],
                          wq[ct * 128:ct * 128 + 128, :])
    if has_bq:
        nc.sync.dma_start(bq_sb[:], bqd[0:128, :])
        nc.sync.dma_start(bq2_sb[:], bqd[128:DH, :])

    def load_wkv():
        for ct in range(CT):
            c0 = ct * 128
            nc.sync.dma_start(wk_sb[:, ct * DH:(ct + 1) * DH],
                              wk[c0:c0 + 128, :])
            nc.sync.dma_start(wv_sb[:, ct * DH:(ct + 1) * DH],
                              wv[c0:c0 + 128, :])
        if has_bk:
            nc.sync.dma_start(bk_sb[:], bkd[0:128, :])
            nc.sync.dma_start(bk2_sb[:], bkd[128:DH, :])

    def load_wo():
        nc.sync.dma_start(wo_sb1[:], wo[0:128, :])
        nc.sync.dma_start(wo_sb2d[0:64, :], wo[128:DH, :])
        nc.sync.dma_start(wo_sb2d[64:128, :], wo[128:DH, :])
        if has_bv:
            for h in range(HPC):
                nc.sync.dma_start(bv_sb[:, h:h + 1],
                                  bvd[h * HD:(h + 1) * HD, :])

    # ---- Phase A: projections interleaved with attention for qb 0 ----
    # PSUM budget (8 banks): pk/pk2/pv 3 projection banks, sc0 2 banks,
    # accA/accB/sumC 3 accumulator banks.
    with (
        tc.tile_pool(name="acc0", bufs=1, space="PSUM") as acc0_pool,
        tc.tile_pool(name="pt0", bufs=6) as pt0_pool,
        tc.tile_pool(name="attnsb", bufs=2) as attnsb,
        tc.tile_pool(name="outsb", bufs=3) as outsb,
      ):
      accA0 = acc0_pool.tile([128, QB], FP32, tag="a0A", name="a0A")
      accB0 = acc0_pool.tile([128, QB], FP32, tag="a0B", name="a0B")
      sumC0 = acc0_pool.tile([128, QB], FP32, tag="a0C", name="a0C")
      with (
        tc.tile_pool(name="stream", bufs=2) as stream,
        tc.tile_pool(name="pproj", bufs=1, space="PSUM") as pproj,
        tc.tile_pool(name="sc0", bufs=2, space="PSUM") as sc0_pool,
      ):
        # q projection (all four query blocks)
        qt2_tiles = []
        for st in range(NQB):
            s0 = st * QB
            ps_q = pproj.tile([128, QB], FP32, tag="pk", name="psq")
            ps_q2 = pproj.tile([64, QB], FP32, tag="pk2", name="psq2")
            if st % 2 == 0:
                qt2_tiles = []
                for ct in range(CT):
                    t = stream.tile([128, 2 * QB], BF16, tag="qt", name="qt",
                                    bufs=12)
                    nc.sync.dma_start(
                        t[:], qTs[ct * 128:(ct + 1) * 128, s0:s0 + 2 * QB])
                    qt2_tiles.append(t)
            qhalf = slice((st % 2) * QB, (st % 2) * QB + QB)
            qt_tiles = [t[:, qhalf] for t in qt2_tiles]
            for ct in range(CT):
                nc.tensor.matmul(
                    ps_q[:], wq_sb[:, ct * DH:ct * DH + 128], qt_tiles[ct][:],
                    start=(ct == 0), stop=(ct == CT - 1))
            for ct in range(CT):
                nc.tensor.matmul(
                    ps_q2[:], wq_sb[:, ct * DH + 128:(ct + 1) * DH],
                    qt_tiles[ct][:],
                    start=(ct == 0), stop=(ct == CT - 1))
            psum_to_sbuf(qhT_pair[:, s0:s0 + QB], ps_q[:],
                         bq_sb[:, 0:1] if has_bq else None)
            psum_to_sbuf(qhT_h2[0:64, s0:s0 + QB], ps_q2[:],
                         bq2_sb[:, 0:1] if has_bq else None)
            nc.sync.dma_start(qhT_h2[64:128, s0:s0 + QB],
                              qhT_h2[0:64, s0:s0 + QB])
            if st == 0:
                load_wkv()

        kt2_tiles = {}
        for kt in range(NKT):
            k0 = kt * KT
            if kt == 2:
                load_wo()
            # k/v loads come in 1024-wide tiles (2KB partition lines);
            # each serves two 512-key superblocks.
            if kt % 2 == 0:
                kw, vw = [], []
                for ct in range(CT):
                    c0 = ct * 128
                    t = stream.tile([128, 2 * KT], BF16, tag="ktile",
                                    name="ktile", bufs=12)
                    nc.sync.dma_start(t[:], kT[c0:c0 + 128, k0:k0 + 2 * KT])
                    kw.append(t)
                    t = stream.tile([128, 2 * KT], BF16, tag="vtile",
                                    name="vtile", bufs=12)
                    nc.sync.dma_start(t[:], vT[c0:c0 + 128, k0:k0 + 2 * KT])
                    vw.append(t)
                kt2_tiles = {"k": kw, "v": vw}
            half = slice((kt % 2) * KT, (kt % 2) * KT + KT)
            kt_tiles = [t[:, half] for t in kt2_tiles["k"]]
            vt_tiles = [t[:, half] for t in kt2_tiles["v"]]
            ps_kh = pproj.tile([128, KT], FP32, tag="pk", name="pskh")
            ps_kh2 = pproj.tile([64, KT], FP32, tag="pk2", name="pskh2")
            for ct in range(CT):
                nc.tensor.matmul(
                    ps_kh[:], wk_sb[:, ct * DH:ct * DH + 128],
                    kt_tiles[ct][:], start=(ct == 0), stop=(ct == CT - 1))
            for ct in range(CT):
                nc.tensor.matmul(
                    ps_kh2[:], wk_sb[:, ct * DH + 128:(ct + 1) * DH],
                    kt_tiles[ct][:], start=(ct == 0), stop=(ct == CT - 1))
            psum_to_sbuf(khT_pair[:, k0:k0 + KT], ps_kh[:],
                         bk_sb[:, 0:1] if has_bk else None)
            psum_to_sbuf(khT_h2[0:64, k0:k0 + KT], ps_kh2[:],
                         bk2_sb[:, 0:1] if has_bk else None)
            nc.sync.dma_start(khT_h2[64:128, k0:k0 + KT],
                              khT_h2[0:64, k0:k0 + KT])
            # v projection for the 4 key blocks of this superblock
            for sj in range(KT // KBLK):
                kb = kt * (KT // KBLK) + sj
                ps_vh = pproj.tile([128, DH], FP32, tag="pv", name="psvh")
                for ct in range(CT):
                    nc.tensor.matmul(
                        ps_vh[:], vt_tiles[ct][:, sj * KBLK:(sj + 1) * KBLK],
                        wv_sb[:, ct * DH:(ct + 1) * DH],
                        start=(ct == 0), stop=(ct == CT - 1))
                ks = slice(kb * HD, (kb + 1) * HD)
                for h in range(HPC):
                    nc.vector.tensor_copy(
                        vh[h][:, ks], ps_vh[:, h * HD:(h + 1) * HD])
            # attention for query block 0, two key blocks at a time
            for sj2 in range(KT // (2 * KBLK)):
                kb0 = kt * (KT // KBLK) + 2 * sj2
                pts = []
                for j in range(2):
                    sc_a = sc0_pool.tile([128, QB], FP32, tag="sc0",
                                         name="sc_a")
                    sc_b = sc0_pool.tile([128, QB], FP32, tag="sc0",
                                         name="sc_b")
                    score_mm(sc_a[:], 0, kb0 + j, 0, QB)
                    score_mm(sc_b[:], 1, kb0 + j, 0, QB)
                    pt = pt0_pool.tile([128, 2 * QB], BF16, tag="pt0",
                                       name="pt0")
                    es.exp(pt[:, 0:QB], sc_a[:])
                    es.exp(pt[:, QB:2 * QB], sc_b[:])
                    pts.append(pt)
                sc_c = sc0_pool.tile([128, QB], FP32, tag="sc0", name="sc_c")
                sc_d = sc0_pool.tile([128, QB], FP32, tag="sc0", name="sc_d")
                score_mm(sc_c[:], 2, kb0, 0, QB, odd=False)
                score_mm(sc_d[:], 2, kb0 + 1, 0, QB, odd=True)
                pt2 = pt0_pool.tile([128, 2 * QB], BF16, tag="pt0",
                                    name="pt2")
                es.exp(pt2[:, 0:QB], sc_c[:])
                es.exp(pt2[:, QB:2 * QB], sc_d[:])
                for j in range(2):
                    kb = kb0 + j
                    attnv(kb,
                          [pts[j][:, 0:QB], pts[j][:, QB:2 * QB],
                           pt2[:, j * QB:(j + 1) * QB]],
                          accA0, accB0, sumC0[64:96, :], 64,
                          first=(kb == 0), last=(kb == NKB - 1),
                          s2_first=(kb == 0), s2_last=(kb == NKB - 1))
      with tc.tile_pool(name="pfin", bufs=2, space="PSUM") as pfin:
        normalize_oproj(0, accA0, accB0, [sumC0[64:96, :]], attnsb, pfin,
                        outsb)

    # ---- Phase B: attention + o-proj for query blocks 1..3 ----
    # PSUM budget (8 banks): sc 2x[128,1024] = 4, accA/accB = 2,
    # scr ring (s2 even/odd sums, then o-proj) = 2.
    with (
        tc.tile_pool(name="scpool", bufs=2, space="PSUM") as scpool,
        tc.tile_pool(name="accpool", bufs=1, space="PSUM") as accpool,
        tc.tile_pool(name="scrpool", bufs=2, space="PSUM") as scrpool,
        tc.tile_pool(name="ptpool", bufs=8) as ptpool,
        tc.tile_pool(name="attnsb", bufs=2) as attnsb,
        tc.tile_pool(name="outsb", bufs=3) as outsb,
    ):
        for qb in range(1, NQB):
            q0 = qb * QB
            accA = accpool.tile([128, QB], FP32, tag="accA", name="accA")
            accB = accpool.tile([128, QB], FP32, tag="accB", name="accB")
            s2e = scrpool.tile([128, QB], FP32, tag="scr", name="s2e")
            s2o = scrpool.tile([128, QB], FP32, tag="scr", name="s2o")
            for kb2 in range(NKB // 2):
                kb0 = 2 * kb2
                sc_h0 = scpool.tile([128, 2 * QB], FP32, tag="sc", name="sch0")
                sc_h1 = scpool.tile([128, 2 * QB], FP32, tag="sc", name="sch1")
                score_mm(sc_h0[:, 0:QB], 0, kb0, q0, QB)
                score_mm(sc_h1[:, 0:QB], 1, kb0, q0, QB)
                score_mm(sc_h0[:, QB:2 * QB], 0, kb0 + 1, q0, QB)
                score_mm(sc_h1[:, QB:2 * QB], 1, kb0 + 1, q0, QB)
                pt_h0 = ptpool.tile([128, 2 * QB], BF16, tag="pt", name="pth0")
                pt_h1 = ptpool.tile([128, 2 * QB], BF16, tag="pt", name="pth1")
                es.exp(pt_h0[:], sc_h0[:])
                es.exp(pt_h1[:], sc_h1[:])
                sc_h2 = scpool.tile([128, 2 * QB], FP32, tag="sc", name="sch2")
                score_mm(sc_h2[:, 0:QB], 2, kb0, q0, QB, odd=False)
                score_mm(sc_h2[:, QB:2 * QB], 2, kb0 + 1, q0, QB, odd=True)
                pt_h2 = ptpool.tile([128, 2 * QB], BF16, tag="pt", name="pth2")
                es.exp(pt_h2[:], sc_h2[:])
                for j in range(2):
                    kb = kb0 + j
                    s2_ap, s2_pos = ((s2e[64:96, :], 64) if j == 0
                                     else (s2o[96:128, :], 96))
                    attnv(kb,
                          [pt_h0[:, j * QB:(j + 1) * QB],
                           pt_h1[:, j * QB:(j + 1) * QB],
                           pt_h2[:, j * QB:(j + 1) * QB]],
                          accA, accB, s2_ap, s2_pos,
                          first=(kb == 0), last=(kb == NKB - 1),
                          s2_first=(kb < 2), s2_last=(kb >= NKB - 2))
            normalize_oproj(q0, accA, accB,
                            [s2e[64:96, :], s2o[96:128, :]],
                            attnsb, scrpool, outsb)


def prepare(q, k, v, Wq, bq, Wk, bk, Wv, bv, Wo, bo):
    """Host-side sharding: returns (in_maps for cores 0-7, bias flags)."""
    bf = ml_dtypes.bfloat16
    qT = np.ascontiguousarray(q[0].T).astype(bf)
    kTf = np.ascontiguousarray(k[0].T).astype(bf)
    vTf = np.ascontiguousarray(v[0].T).astype(bf)
    wqT = np.ascontiguousarray(np.asarray(Wq).T).astype(bf)
    wkT = np.ascontiguousarray(np.asarray(Wk).T).astype(bf)
    wvT = np.ascontiguousarray(np.asarray(Wv).T).astype(bf)
    woT = np.ascontiguousarray(np.asarray(Wo).T).astype(bf)
    bq = np.asarray(bq, np.float32)
    bk = np.asarray(bk, np.float32)
    bv = np.asarray(bv, np.float32)
    in_maps = []
    for core in range(8):
        g, s = divmod(core, 2)
        d0, d1 = g * DH, (g + 1) * DH
        in_maps.append({
            "qTs": np.ascontiguousarray(qT[:, s * SQ:(s + 1) * SQ]),
            "kT": kTf,
            "vT": vTf,
            "wq": np.ascontiguousarray(wqT[:, d0:d1]),
            "wk": np.ascontiguousarray(wkT[:, d0:d1]),
            "wv": np.ascontiguousarray(wvT[:, d0:d1]),
            "wo": np.ascontiguousarray(woT[d0:d1, :]),
            "bq": np.ascontiguousarray(bq[d0:d1]).reshape(DH, 1),
            "bk": np.ascontiguousarray(bk[d0:d1]).reshape(DH, 1),
            "bv": np.ascontiguousarray(bv[d0:d1]).reshape(DH, 1),
        })
    flags = (bool(np.any(bq)), bool(np.any(bk)), bool(np.any(bv)))
    return in_maps, flags


def combine(results, bo):
    """Host-side unsharding: sum o-proj partials per half, concat, add bo."""
    halves = []
    for s in range(2):
        acc = None
        for g in range(4):
            o = np.asarray(results[g * 2 + s]["outT"], np.float32)
            acc = o if acc is None else acc + o
        halves.append(acc.T)
    out = np.concatenate(halves, axis=0) + np.asarray(bo, np.float32)
    return np.ascontiguousarray(out).reshape(1, SEQ, D).astype(np.float32)


def kernel(q, k, v, Wq, bq, Wk, bk, Wv, bv, Wo, bo):
    from concourse.bass_utils import run_bass_kernel_spmd

    in_maps, flags = prepare(q, k, v, Wq, bq, Wk, bk, Wv, bv, Wo, bo)
    nc = build_program(*flags)
    last_err = None
    for _attempt in range(3):
        try:
            res = run_bass_kernel_spmd(nc, in_maps, list(range(8)))
            return combine(res.results, bo)
        except Exception as e:  # transient NRT/device wedges recover on retry
            last_err = e
            try:
                import jax
                jax.clear_caches()
                jax.extend.backend.clear_backends()
            except Exception:
                pass
    raise last_err
